# revision 1
# baseline (speedup 1.0000x reference)
"""BTT layer (nn_BTTLayer_36885179138559) as a Trainium2 Bass kernel.

Math: out = x @ W + bias where W[n*64+b, m*64+a] = sum_r btt_r[n,b,m*8+r] *
btt_l[m, n*8+r, a]  (the BTT two-stage contraction collapses to one dense
4096x4096 matmul; W is precomputed on host from the small BTT cores).

Sharding: data-parallel over the flattened batch (4096 rows) across 8
NeuronCores, 512 rows each; W replicated. On-device compute in bf16
(fp32 PSUM accumulation), out returned fp32.
"""

import numpy as np
import ml_dtypes

import concourse.bacc as bacc
import concourse.mybir as mybir
import concourse.tile as tile
import concourse.bass_utils as bass_utils

# problem dims (hardcoded per contract)
M, N, A, B_BLK, RANK = 64, 64, 64, 64, 8
D = 4096              # in = out features
ROWS = 4096           # flattened batch (4, 1024, 4096)
N_CORES = 8
BS = ROWS // N_CORES  # 512 rows per core
KT = 32               # k tiles of 128
OC = 8                # out-column tiles of 512
BT = 4                # batch tiles of 128

BF16 = mybir.dt.bfloat16
F32 = mybir.dt.float32

_compiled = None
_last_in_maps = None


def _build():
    nc = bacc.Bacc("TRN2", target_bir_lowering=False, debug=False, num_devices=N_CORES)
    xt_ap = nc.dram_tensor("xt", [128, KT * BS], BF16, kind="ExternalInput").ap()
    w_ap = nc.dram_tensor("w", [OC, 128, KT * 512], BF16, kind="ExternalInput").ap()
    o_ap = nc.dram_tensor("o", [OC, BT, 128, 512], F32, kind="ExternalOutput").ap()

    X_SPLIT = [2] * 16                           # kt per x tile (sums to 32)
    W_SPLIT0 = [4] * 8                           # kt per W sub-tile per slab
    W_SPLIT = [4] * 8
    with tile.TileContext(nc) as tc:
        with (
            tc.tile_pool(name="xpool", bufs=1) as xpool,
            tc.tile_pool(name="wpool", bufs=18) as wpool,
            tc.tile_pool(name="opool", bufs=4) as opool,
            tc.tile_pool(name="psum", bufs=6, space="PSUM") as psum,
        ):
            xg_tiles = []      # per kt: (tile, kt-offset within tile)
            kt2x = []
            off = 0
            for g, nkt in enumerate(X_SPLIT):
                Xg = xpool.tile([128, nkt * BS], BF16, tag=f"x{g}", name=f"X{g}")
                nc.sync.dma_start(Xg[:], xt_ap[:, off * BS:(off + nkt) * BS])
                for j in range(nkt):
                    kt2x.append((Xg, j))
                off += nkt
                xg_tiles.append(Xg)
            for oc in range(OC):
                wsplit = W_SPLIT0 if oc == 0 else W_SPLIT
                kt2w = []      # per kt: (sub tile, kt-offset within sub)
                woffk = 0
                for s, nkt in enumerate(wsplit):
                    Wsub = wpool.tile([128, nkt * 512], BF16, tag="w", name=f"W{oc}_{s}")
                    nc.scalar.dma_start(
                        Wsub[:], w_ap[oc][:, woffk * 512:(woffk + nkt) * 512]
                    )
                    for j in range(nkt):
                        kt2w.append((Wsub, j))
                    woffk += nkt
                ps_list = [
                    psum.tile([128, 512], F32, tag="ps", name=f"ps_{oc}_{bt}")
                    for bt in range(BT)
                ]
                if oc < OC - 1:
                    # kt-outer: all 4 psum banks accumulate in lockstep so each
                    # W sub-tile feeds 4x the matmuls -> W stream never stalls PE
                    for kt in range(KT):
                        Xg, xj = kt2x[kt]
                        Wsub, wj = kt2w[kt]
                        for bt in range(BT):
                            xoff = xj * BS + bt * 128
                            nc.tensor.matmul(
                                ps_list[bt][:],
                                Xg[:, xoff:xoff + 128],
                                Wsub[:, wj * 512:(wj + 1) * 512],
                                start=(kt == 0),
                                stop=(kt == KT - 1),
                            )
                    for bt in range(BT):
                        osb = opool.tile([128, 512], F32, tag="o")
                        nc.scalar.copy(osb[:], ps_list[bt][:])
                        nc.sync.dma_start(o_ap[oc, bt], osb[:])
                else:
                    # last slab: bt-outer so evict+store of early bt tiles
                    # overlaps the remaining matmuls (W is fully prefetched here)
                    for bt in range(BT):
                        for kt in range(KT):
                            Xg, xj = kt2x[kt]
                            Wsub, wj = kt2w[kt]
                            xoff = xj * BS + bt * 128
                            nc.tensor.matmul(
                                ps_list[bt][:],
                                Xg[:, xoff:xoff + 128],
                                Wsub[:, wj * 512:(wj + 1) * 512],
                                start=(kt == 0),
                                stop=(kt == KT - 1),
                            )
                        osb = opool.tile([128, 512], F32, tag="o")
                        nc.scalar.copy(osb[:], ps_list[bt][:])
                        nc.sync.dma_start(o_ap[oc, bt], osb[:])
    nc.compile()
    return nc


def _get_compiled():
    global _compiled
    if _compiled is None:
        _compiled = _build()
    return _compiled


def kernel(x, btt_r, btt_l, bias):
    x = np.asarray(x)
    btt_r = np.asarray(btt_r)
    btt_l = np.asarray(btt_l)
    bias = np.asarray(bias)
    orig_shape = x.shape

    # ---- host: collapse BTT cores into dense W (fp32) ----
    r4 = btt_r.astype(np.float32).reshape(N, B_BLK, M, RANK)      # [n, b, m, r]
    l4 = btt_l.astype(np.float32).reshape(M, N, RANK, A)          # [m, n, r, a]
    # W[n, b, m, a] = sum_r r4[n,b,m,r] * l4[m,n,r,a]
    W = np.einsum("nbmr,mnra->nbma", r4, l4, optimize=True)
    W = W.reshape(D, D)

    # device W layout: (OC, 128, KT*512); W_dev[oc, kp, kt*512+c] = W[kt*128+kp, oc*512+c]
    W_dev = np.ascontiguousarray(
        W.reshape(KT, 128, OC, 512).transpose(2, 1, 0, 3).reshape(OC, 128, KT * 512)
    ).astype(ml_dtypes.bfloat16)

    # per-core x shards, transposed: X_dev[kp, kt*BS + col] = xs[col, kt*128+kp]
    xr = x.astype(np.float32).reshape(ROWS, D)
    in_maps = []
    for c in range(N_CORES):
        xs = xr[c * BS:(c + 1) * BS]                               # (BS, D)
        xt = np.ascontiguousarray(
            xs.T.reshape(KT, 128, BS).transpose(1, 0, 2).reshape(128, KT * BS)
        ).astype(ml_dtypes.bfloat16)
        in_maps.append({"xt": xt, "w": W_dev})

    global _last_in_maps
    _last_in_maps = in_maps
    nc = _get_compiled()
    try:
        res = bass_utils.run_bass_kernel_spmd(nc, in_maps, core_ids=list(range(N_CORES)))
    except Exception:
        # transient device hiccups recover on retry
        import time as _time
        _time.sleep(10)
        res = bass_utils.run_bass_kernel_spmd(nc, in_maps, core_ids=list(range(N_CORES)))

    # ---- gather: o (OC, BT, 128, 512) -> rows (BS, D) per core ----
    out = np.empty((ROWS, D), dtype=np.float32)
    for c in range(N_CORES):
        o = np.asarray(res.results[c]["o"], dtype=np.float32)       # (OC, BT, 128, 512)
        # out[bt*128+p, oc*512+c2] = o[oc, bt, p, c2]
        out[c * BS:(c + 1) * BS] = o.transpose(1, 2, 0, 3).reshape(BS, D)
    out += bias.astype(np.float32)[None, :]
    return out.reshape(*orig_shape[:-1], D)



# revision 10
# speedup vs baseline: 1.0977x; 1.0977x over previous
"""BTT layer (nn_BTTLayer_36885179138559) as a Trainium2 Bass kernel.

Factorized BTT evaluation (no dense-W collapse): per core (data-parallel over
512 of the 4096 flattened batch rows),
  stage 1: inner[n, B, m*8+r] = x_n[B, 64] @ btt_r[n]        (64 matmuls, K=64)
  butterfly: T[(n,r), B] per (m, n-group) via PE transpose with a gathered
             access pattern (the BTT block permutation)
  stage 2: out[B, m*64+a] = sum_g T_slab.T @ btt_l[m, g-slab] (PSUM accum)
4x fewer FLOPs than the dense-W kernel; weights traffic 8MB vs 32MB.
Compute in bf16 (fp32 PSUM), device output bf16, host casts to fp32.
"""

import numpy as np
import ml_dtypes

import concourse.bacc as bacc
import concourse.mybir as mybir
import concourse.tile as tile
import concourse.bass_utils as bass_utils

# problem dims (hardcoded per contract)
M, N, A, B_BLK, RANK = 64, 64, 64, 64, 8
D = 4096              # in = out features
ROWS = 4096           # flattened batch (4, 1024, 4096)
N_CORES = 8
BS = ROWS // N_CORES  # 512 rows per core
BT = 4                # batch tiles of 128

BF16 = mybir.dt.bfloat16
F32 = mybir.dt.float32

_compiled = None
_last_in_maps = None


def _build():
    nc = bacc.Bacc("TRN2", target_bir_lowering=False, debug=False, num_devices=N_CORES)
    # xt/rt: [128, 32, 512]; tile t holds feature rows 128t..128t+128
    # (two 64-row n-blocks per tile), columns = batch rows (xt) / m*8+r (rt)
    xt_ap = nc.dram_tensor("xt", [128, 32, 512], BF16, kind="ExternalInput").ap()
    rt_ap = nc.dram_tensor("rt", [128, 32, 512], BF16, kind="ExternalInput").ap()
    # lt: [128, m, g, a] = btt_l[m, (16g + p//8)*8 + p%8, a]
    lt_ap = nc.dram_tensor("lt", [128, M, 4, A], BF16, kind="ExternalInput").ap()
    id_ap = nc.dram_tensor("ident", [128, 128], BF16, kind="ExternalInput").ap()
    # o: [bt, 128, m*64+a] bf16
    o_ap = nc.dram_tensor("o", [BT, 128, D], BF16, kind="ExternalOutput").ap()

    with tile.TileContext(nc) as tc:
        with (
            tc.tile_pool(name="xin", bufs=1) as xin,
            tc.tile_pool(name="spool", bufs=1) as spool,
            tc.tile_pool(name="tpool", bufs=4) as tpool,
            tc.tile_pool(name="opool", bufs=2) as opool,
            tc.tile_pool(name="ps1p", bufs=2, space="PSUM") as ps1p,
            tc.tile_pool(name="psTp", bufs=2, space="PSUM") as psTp,
            tc.tile_pool(name="ps2p", bufs=2, space="PSUM") as ps2p,
        ):
            ID = xin.tile([128, 128], BF16, tag="id", name="ID")
            nc.sync.dma_start(ID[:], id_ap)
            # x/r in 4 chunks of 8 tiles so stage 1 can start early
            XT = xin.tile([128, 32, 512], BF16, tag="xt", name="XT")
            RT = xin.tile([128, 32, 512], BF16, tag="rt", name="RT")
            for q in range(4):
                nc.sync.dma_start(XT[:, 8 * q:8 * (q + 1), :], xt_ap[:, 8 * q:8 * (q + 1), :])
                nc.sync.dma_start(RT[:, 8 * q:8 * (q + 1), :], rt_ap[:, 8 * q:8 * (q + 1), :])
            LT = xin.tile([128, M, 4, A], BF16, tag="lt", name="LT")
            nc.sync.dma_start(LT[:], lt_ap)

            # inner for one batch tile: S[p=row, m, n*8+r] — m-major free
            # layout so each butterfly transpose reads 128 contiguous cols
            S = spool.tile([128, M, 512], BF16, tag="S", name="S")

            for bt in range(BT):
                # ---- stage 1: 64 (K=64) matmuls, paired for batched evict ----
                for np_ in range(32):
                    ps1 = ps1p.tile([128, 2, 512], F32, tag="ps1", name=f"ps1_{bt}_{np_}")
                    for h in range(2):
                        n = 2 * np_ + h
                        pl, ph = 64 * h, 64 * (h + 1)
                        nc.tensor.matmul(
                            ps1[:, h, :],
                            XT[pl:ph, np_, 128 * bt:128 * (bt + 1)],
                            RT[pl:ph, np_, :],
                            start=True, stop=True,
                        )
                    # f32 psum -> bf16 sbuf, one [128, 1024] instr; scatter the
                    # n-pair's (m, r) block into S's m-major layout. The S
                    # write AP is a contiguous reshape (dep-tracker safe);
                    # the reordered traversal lives on the psum read side.
                    # (GPSIMD/Pool cannot read PSUM; split Act 3:1 DVE)
                    src = ps1[:].rearrange("p h (m r) -> p m h r", m=M, r=RANK)
                    dst = S[:, :, 16 * np_:16 * (np_ + 1)].rearrange(
                        "p m (h r) -> p m h r", h=2, r=RANK)
                    if np_ % 4 == 3:
                        nc.vector.tensor_copy(dst, src)
                    else:
                        nc.scalar.copy(dst, src)

                # ---- butterfly + stage 2, s2 lagged one m-pair behind ----
                tslabs = {}
                for mp in range(33):
                    if mp < 32:
                        psT = psTp.tile([128, 8, 128], BF16, tag="psT", name=f"psT_{bt}_{mp}")
                        for h in range(2):
                            m = 2 * mp + h
                            for g in range(4):
                                nc.tensor.transpose(
                                    psT[:, 4 * h + g, :],
                                    S[:, m, 128 * g:128 * (g + 1)],
                                    ID[:],
                                )
                        Ts = tpool.tile([128, 8, 128], BF16, tag="T", name=f"T_{bt}_{mp}")
                        nc.vector.tensor_copy(Ts[:], psT[:])
                        tslabs[mp] = Ts
                    if mp >= 1:
                        Ts = tslabs.pop(mp - 1)
                        for h in range(2):
                            m = 2 * (mp - 1) + h
                            q = m % 8
                            if q == 0:
                                ps2 = ps2p.tile([128, 8, A], F32, tag="ps2", name=f"ps2_{bt}_{m}")
                            if m == 0:
                                OutSB = opool.tile([128, M, A], BF16, tag="o", name=f"O_{bt}")
                            for g in range(4):
                                nc.tensor.matmul(
                                    ps2[:, q, :],
                                    Ts[:, 4 * h + g, :],
                                    LT[:, m, g, :],
                                    start=(g == 0), stop=(g == 3),
                                )
                            if q == 7:
                                nc.scalar.copy(OutSB[:, m - 7:m + 1, :], ps2[:])
                nc.sync.dma_start(o_ap[bt], OutSB[:])
    nc.compile()
    return nc


def _get_compiled():
    global _compiled
    if _compiled is None:
        _compiled = _build()
    return _compiled


def kernel(x, btt_r, btt_l, bias):
    x = np.asarray(x)
    btt_r = np.asarray(btt_r)
    btt_l = np.asarray(btt_l)
    bias = np.asarray(bias)
    orig_shape = x.shape

    # rt[p, t, j] = btt_r[2t + p//64, p%64, j]
    rt = np.ascontiguousarray(
        btt_r.astype(np.float32).reshape(32, 2, 64, 512).transpose(1, 2, 0, 3)
        .reshape(128, 32, 512)
    ).astype(ml_dtypes.bfloat16)
    # lt[p, m, g, a] = btt_l[m, (16g + p//8)*8 + p%8, a] = l4[m, 16g + p//8, p%8, a]
    l4 = btt_l.astype(np.float32).reshape(M, 4, 16, RANK, A)      # [m, g, nloc, r, a]
    lt = np.ascontiguousarray(l4.transpose(2, 3, 0, 1, 4).reshape(128, M, 4, A)
                              ).astype(ml_dtypes.bfloat16)
    ident = np.eye(128, dtype=ml_dtypes.bfloat16)

    # per-core x shards, transposed: xt[p, t, col] = xs[col, 128t + p]
    xr = x.astype(np.float32).reshape(ROWS, D)
    in_maps = []
    for c in range(N_CORES):
        xs = xr[c * BS:(c + 1) * BS]                               # (BS, D)
        xt = np.ascontiguousarray(
            xs.T.reshape(32, 128, BS).transpose(1, 0, 2)
        ).astype(ml_dtypes.bfloat16)
        in_maps.append({"xt": xt, "rt": rt, "lt": lt, "ident": ident})

    global _last_in_maps
    _last_in_maps = in_maps
    nc = _get_compiled()
    try:
        res = bass_utils.run_bass_kernel_spmd(nc, in_maps, core_ids=list(range(N_CORES)))
    except Exception:
        # transient device hiccups recover on retry
        import time as _time
        _time.sleep(10)
        res = bass_utils.run_bass_kernel_spmd(nc, in_maps, core_ids=list(range(N_CORES)))

    # ---- gather: o (BT, 128, D) bf16 -> rows (BS, D) per core ----
    out = np.empty((ROWS, D), dtype=np.float32)
    for c in range(N_CORES):
        o = np.asarray(res.results[c]["o"], dtype=np.float32)       # (BT, 128, D)
        out[c * BS:(c + 1) * BS] = o.reshape(BS, D)
    out += bias.astype(np.float32)[None, :]
    return out.reshape(*orig_shape[:-1], D)
